# revision 67
# baseline (speedup 1.0000x reference)
"""NT-Xent loss kernel, v20: symmetric-block decomposition (36.9us
cost-model vs 47.9us baseline; rel err ~5e-3).

v20 vs v19: sims land in FOUR independent single-buffer PSUM granule
rings per q-tile (A=b0,b1 [128,1024]; B1=b2, B2=b3, C=b4 [128,512]
each, 5 banks total) instead of two. Each granule gets its own
exp+accum into a distinct output column (host sums the four partial
row-sum groups). Finer rings decouple PE from ACT: the next strip's
sims for one 512-col block only wait for THAT block's exp, so the
exp->sims->exp serialization cycle shrinks and ACT runs ~69%% busy
through the endgame. W matmuls are issued after the chunk loop
(floor 12us) to fill the remaining PE gaps.

Decomposition: after the host-side rotation every core's q rows are
local rows 0-511; each core computes sim blocks (q, B0..B4) = local
cols 0..2559 only (5 of 8 column blocks). Row sums come from the exp
accumulators; the missing col groups m=5,6,7 for each row are COLUMN
sums of blocks B3,B2,B1 computed on cores i+5,i+6,i+7 (sim symmetry),
taken with ones-vector matmuls over the retained f32r exp values and
combined on the host. B4 pairs with core i+4's B4 (computed twice
fleet-wide, row sums only). This cuts sim matmuls 13.7->8.5us on PE
and exp work 17->11.5us on ACT vs the full-row v11 design.

Schedule/assignment (found by cost-model iteration):
- 4-rows-per-partition interleave: partition p of tile t=4ch+k holds
  local row 512ch+4p+k -> dist DMA descriptors are 1600B contiguous
  (2.28us vs 4.55us) and q/dist tiles line up for the P path.
- norms: one scalar_tensor_tensor (x*1)*x with accum_out per tile,
  all on DVE (GPSIMD rejects generic tensor ops at codegen).
- rn = exp(-0.5 ln norm2) on ACT; nn = norm2*rn on DVE.
- scale rows 0..2559 in place: ch0-1 all-Pool normalize_recip; ch2-4
  k0,k1 via one DVE broadcast-STT + k2,k3 Pool. Rows 2560+ instead
  fold rn into dist (sdist, Pool nrecip); W uses raw z there.
- transposes on PE; PSUM->SBUF copies on ACT.
- sims land in PSUM granules A=[128,1024] (b0,b1) + B=[128,1536]
  (b2-b4) per q-tile, ONE exp+accum each straight into the output
  tile; granule pools let PE run ahead of ACT.
- W = dist^T z_hat (32 matmuls) issued after the chunk loop to fill
  PE gaps during the exp phase; P path q_ps padded to 256 free
  (1 cyc/row) with the mask-multiply+reduce fused into one STT.
- colsum matmuls accumulate over the 4 q-tiles in a 3-bank PSUM
  window after the W pool closes; staged to SBUF and DMA'd as out2.
- PE warm-up matmuls ramp the tensor clock before the transposes.
- tile_set_cur_wait floors stage the scheduler per chunk arrival
  (DMA sem overhead ~0.9us included).

Engine legality learned on hardware: GPSIMD supports only its custom
ISA ops (normalize_recip etc) and cannot access PSUM; activation table
set 6 = {ln, exp, square, copy, ...}; Rsqrt/Reciprocal are banned;
f32r matmul inputs must be produced typed f32r (DVE/ACT writes
qualify); transpose-mode matmul needs a permutation rhs;
tensor_tensor_reduce crashes the exec unit.
"""

import numpy as np
from contextlib import ExitStack

N = 4096
D = 256
C = 100
B = 2048
N_CORES = 8
RPC = 512
NT = 32
NCH = 8
NBLK = 5
SIMW = NBLK * 512
NSC = NBLK * 4
E_CONST = float(np.e)

_PROG = None


def _build_program():
    import concourse.bass as bass
    import concourse.tile as tile
    from concourse import bacc, mybir, masks

    f32 = mybir.dt.float32
    f32r = mybir.dt.float32r
    MULT = mybir.AluOpType.mult
    EXP = mybir.ActivationFunctionType.Exp
    LN = mybir.ActivationFunctionType.Ln

    nc = bacc.Bacc(
        "TRN2",
        target_bir_lowering=False,
        debug=False,
        enable_asserts=False,
        num_devices=N_CORES,
    )

    z = nc.dram_tensor("z", [N, D], f32r, kind="ExternalInput").ap()
    dist = nc.dram_tensor("dist", [B, C], f32r, kind="ExternalInput").ap()
    out = nc.dram_tensor("out", [128, 20], f32, kind="ExternalOutput").ap()
    out2 = nc.dram_tensor("out2", [1, 1536], f32, kind="ExternalOutput").ap()

    with tile.TileContext(nc) as tc, ExitStack() as ctx:
        per = ctx.enter_context(tc.tile_pool(name="persist", bufs=1))

        nc.scalar.add_instruction(mybir.InstLoadActFuncSet(
            name=f"I-{nc.next_id()}", ins=[], outs=[], act_func_set_id=6))

        ident = per.tile([128, 128], f32)
        masks.make_identity(nc, ident[:])
        ident_r = per.tile([128, 128], f32r)
        nc.vector.tensor_copy(out=ident_r[:], in_=ident[:])
        ones_f = per.tile([128, 1], f32)
        nc.vector.memset(ones_f[:], 1.0)
        ones_r = per.tile([128, 1], f32r)
        nc.vector.tensor_copy(out=ones_r[:], in_=ones_f[:])

        zn = per.tile([128, NT * 256], f32r)
        zT = per.tile([128, 2 * SIMW], f32r)
        exp_sb = per.tile([128, 4 * SIMW], f32r)
        dist_sb = per.tile([128, 4 * 400], f32r)
        sdist = per.tile([128, 12 * 100], f32r)
        norm2 = per.tile([128, NT], f32)
        lnn = per.tile([128, NT], f32)
        nn = per.tile([128, NT], f32)
        rn = per.tile([128, NT], f32)
        wt_sb = per.tile([128, 256], f32)
        w_sb = per.tile([128, 2 * 256], f32r)
        junk = per.tile([128, C], f32)
        out_sb = per.tile([128, 20], f32)
        cs_sb = per.tile([1, 1536], f32)
        nc.vector.memset(w_sb[:].bitcast(f32), 0.0)

        # ---------------- DMAs: 8 z chunks, then dist ----------------------
        for ch in range(NCH):
            nc.sync.dma_start(
                out=zn[:, ch * 1024:(ch + 1) * 1024].rearrange(
                    "p (q j) -> p q j", j=256),
                in_=z[ch * 512:(ch + 1) * 512, :].rearrange(
                    "(p q) j -> p q j", q=4),
            )
        nc.sync.dma_start(
            out=dist_sb[:].rearrange("p (ch q c) -> p ch q c", q=4, c=C),
            in_=dist.rearrange("(ch p q) c -> p ch q c", p=128, q=4),
        )

        # ---------------- PE warm-up: ramp tensor clock --------------------
        with tc.tile_pool(name="warm", bufs=1, space="PSUM") as wp:
            warm = wp.tile([128, 128], f32, tag="warm")
            for _ in range(16):
                nc.tensor.matmul(warm[:], lhsT=ident_r[:], rhs=ident_r[:],
                                 start=True, stop=True)

        sqd = ctx.enter_context(tc.tile_pool(name="sqd", bufs=3))

        def arrival(ch):
            return 2.9 + 1.46 * ch

        def norms_chunk(ch):
            for k in range(4):
                t = 4 * ch + k
                src = zn[:, t * 256:(t + 1) * 256].bitcast(f32)
                s = sqd.tile([128, 256], f32, tag="sq", name=f"sq{t}")
                nc.vector.scalar_tensor_tensor(
                    out=s[:], in0=src, scalar=1.0, in1=src,
                    op0=MULT, op1=MULT,
                    accum_out=norm2[:, t:t + 1])

        def rn_chain(t0, nt, need_nn):
            sl = slice(t0, t0 + nt)
            with tc.high_priority():
                nc.scalar.activation(lnn[:, sl], norm2[:, sl], LN)
                nc.scalar.activation(rn[:, sl], lnn[:, sl], EXP, scale=-0.5)
                if need_nn:
                    nc.vector.tensor_tensor(out=nn[:, sl], in0=norm2[:, sl],
                                            in1=rn[:, sl], op=MULT)

        def scale_chunk(ch):
            t0 = 4 * ch
            if ch >= 2:
                dst = zn[:, t0 * 256:(t0 + 2) * 256]
                bc = rn[:, t0:t0 + 2].unsqueeze(2).to_broadcast([128, 2, 256])
                nc.vector.scalar_tensor_tensor(
                    out=dst.rearrange("p (k j) -> p k j", j=256),
                    in0=dst.bitcast(f32).rearrange("p (k j) -> p k j", j=256),
                    scalar=1.0, in1=bc, op0=MULT, op1=MULT)
                ks = (2, 3)
            else:
                ks = (0, 1, 2, 3)
            for k in ks:
                t = 4 * ch + k
                d2 = zn[:, t * 256:(t + 1) * 256]
                nc.gpsimd.normalize_recip(
                    out_ap=d2, in_ap=d2.bitcast(f32),
                    denom_ap=nn[:, t:t + 1])

        def transpose_chunk(ptp, ch):
            for d in range(2):
                pt = ptp.tile([128, 512], f32r, tag="tr", name=f"pt{ch}_{d}")
                for k in range(4):
                    t = 4 * ch + k
                    nc.tensor.transpose(
                        pt[:, k * 128:(k + 1) * 128],
                        zn[:, t * 256 + d * 128: t * 256 + d * 128 + 128],
                        ident_r[:],
                    )
                dst = zT[:, d * SIMW + ch * 512: d * SIMW + (ch + 1) * 512]
                nc.scalar.copy(out=dst, in_=pt[:])

        def w_tiles(ts):
            for t in ts:
                if t < NSC:
                    lhsT = dist_sb[:, (t // 4 % 4) * 400 + (t % 4) * 100:
                                   (t // 4 % 4) * 400 + (t % 4) * 100 + 100]
                else:
                    lhsT = sdist[:, (t - NSC) * 100:(t - NSC + 1) * 100]
                nc.tensor.matmul(
                    wt_ps[0:C, :], lhsT=lhsT,
                    rhs=zn[:, t * 256:(t + 1) * 256],
                    start=(t == 0), stop=(t == NT - 1),
                )

        gA = ctx.enter_context(tc.tile_pool(name="gA", bufs=1, space="PSUM"))
        gB = ctx.enter_context(tc.tile_pool(name="gB", bufs=1, space="PSUM"))
        gC = ctx.enter_context(tc.tile_pool(name="gC", bufs=1, space="PSUM"))
        gB2 = ctx.enter_context(tc.tile_pool(name="gB2", bufs=1, space="PSUM"))
        pw_cm = tc.tile_pool(name="pw", bufs=1, space="PSUM")
        pw = pw_cm.__enter__()
        wt_ps = pw.tile([128, 256], f32, tag="wt", name="wt_ps")

        def sims_into(g, rt, b, col0):
            for d in range(2):
                nc.tensor.matmul(
                    g[:, col0:col0 + 512],
                    lhsT=zT[:, d * SIMW + rt * 128: d * SIMW + (rt + 1) * 128],
                    rhs=zT[:, d * SIMW + b * 512: d * SIMW + (b + 1) * 512],
                    start=(d == 0), stop=(d == 1),
                )

        GOFF = {0: 0, 1: 1024, 2: 1536, 3: 2048}

        def exp_granule(g, rt, gi, width):
            nc.scalar.activation(
                out=exp_sb[:, rt * SIMW + GOFF[gi]:
                           rt * SIMW + GOFF[gi] + width],
                in_=g[:, 0:width], func=EXP,
                accum_out=out_sb[:, gi * 4 + rt: gi * 4 + rt + 1])

        def granules_for(rt):
            a = gA.tile([128, 1024], f32, tag="gA", name=f"g{rt}a")
            sims_into(a, rt, 0, 0)
            sims_into(a, rt, 1, 512)
            exp_granule(a, rt, 0, 1024)
            bb = gB.tile([128, 512], f32, tag="gB", name=f"g{rt}b")
            sims_into(bb, rt, 2, 0)
            exp_granule(bb, rt, 1, 512)
            b2 = gB2.tile([128, 512], f32, tag="gB2", name=f"g{rt}b2")
            sims_into(b2, rt, 3, 0)
            exp_granule(b2, rt, 2, 512)
            cc = gC.tile([128, 512], f32, tag="gC", name=f"g{rt}c")
            sims_into(cc, rt, 4, 0)
            exp_granule(cc, rt, 3, 512)

        # ---------------- chunks 0-4 + strip rt0 granules -------------------
        with tc.tile_pool(name="pt", bufs=2, space="PSUM") as ptp:
            a0 = b0 = None
            for ch in range(NBLK):
                tc.tile_set_cur_wait(arrival(ch) / 1000.0)
                norms_chunk(ch)
                tc.tile_set_cur_wait((arrival(ch) + 0.40) / 1000.0)
                rn_chain(4 * ch, 4, need_nn=True)
                tc.tile_set_cur_wait((arrival(ch) + 0.75) / 1000.0)
                scale_chunk(ch)
                tc.tile_set_cur_wait((arrival(ch) + 1.30) / 1000.0)
                transpose_chunk(ptp, ch)
                tc.tile_set_cur_wait((arrival(ch) + 1.95) / 1000.0)
                if ch == 0:
                    a0 = gA.tile([128, 1024], f32, tag="gA", name="g0a")
                    sims_into(a0, 0, 0, 0)
                elif ch == 1:
                    sims_into(a0, 0, 1, 512)
                    exp_granule(a0, 0, 0, 1024)
                elif ch == 2:
                    b0 = gB.tile([128, 512], f32, tag="gB", name="g0b")
                    sims_into(b0, 0, 2, 0)
                    exp_granule(b0, 0, 1, 512)
                elif ch == 3:
                    b02 = gB2.tile([128, 512], f32, tag="gB2", name="g0b2")
                    sims_into(b02, 0, 3, 0)
                    exp_granule(b02, 0, 2, 512)
                else:
                    c0 = gC.tile([128, 512], f32, tag="gC", name="g0c")
                    sims_into(c0, 0, 4, 0)
                    exp_granule(c0, 0, 3, 512)

        for ch in range(NBLK, NCH):
            tc.tile_set_cur_wait(arrival(ch) / 1000.0)
            norms_chunk(ch)
        tc.tile_set_cur_wait((arrival(7) + 0.40) / 1000.0)
        rn_chain(NSC, 12, need_nn=True)
        for t in range(NSC, NT):
            srcd = dist_sb[:, (t // 4 % 4) * 400 + (t % 4) * 100:
                           (t // 4 % 4) * 400 + (t % 4) * 100 + 100]
            nc.gpsimd.normalize_recip(
                out_ap=sdist[:, (t - NSC) * 100:(t - NSC + 1) * 100],
                in_ap=srcd.bitcast(f32),
                denom_ap=nn[:, t:t + 1])

        # ---------------- strips 1-3, W tail, P path ------------------------
        tc.tile_set_cur_wait(12.0 / 1000.0)
        w_tiles(range(0, NSC))
        granules_for(1)
        tc.tile_set_cur_wait((arrival(7) + 1.5) / 1000.0)
        w_tiles(range(NSC, NT))
        granules_for(2)

        nc.vector.tensor_copy(out=wt_sb[0:C, :], in_=wt_ps[0:C, :])
        for d in range(2):
            w_ps = pw.tile([128, 128], f32, tag="wt", name=f"w_ps{d}")
            nc.tensor.transpose(
                w_ps[:, 0:C],
                wt_sb[0:C, d * 128:(d + 1) * 128],
                ident[0:C, 0:C],
            )
            nc.vector.tensor_copy(out=w_sb[:, d * 256:d * 256 + C],
                                  in_=w_ps[:, 0:C])

        cc3 = gC.tile([128, 512], f32, tag="gC", name="g3c")
        sims_into(cc3, 3, 4, 0)
        exp_granule(cc3, 3, 3, 512)
        bb3 = gB.tile([128, 512], f32, tag="gB", name="g3b")
        sims_into(bb3, 3, 2, 0)
        exp_granule(bb3, 3, 1, 512)
        b32 = gB2.tile([128, 512], f32, tag="gB2", name="g3b2")
        sims_into(b32, 3, 3, 0)
        exp_granule(b32, 3, 2, 512)
        a3 = gA.tile([128, 1024], f32, tag="gA", name="g3a")
        sims_into(a3, 3, 0, 0)
        sims_into(a3, 3, 1, 512)
        exp_granule(a3, 3, 0, 1024)

        for rt in range(4):
            q_ps = pw.tile([128, 256], f32, tag="wt", name=f"q_ps{rt}")
            for d in range(2):
                nc.tensor.matmul(
                    q_ps[:], lhsT=zT[:, d * SIMW + rt * 128:
                                     d * SIMW + (rt + 1) * 128],
                    rhs=w_sb[:, d * 256:(d + 1) * 256],
                    start=(d == 0), stop=(d == 1),
                )
            nc.vector.scalar_tensor_tensor(
                out=junk[:], in0=q_ps[:, 0:C], scalar=1.0,
                in1=dist_sb[:, rt * 100:rt * 100 + 100].bitcast(f32),
                op0=MULT, op1=MULT,
                accum_out=out_sb[:, 16 + rt:17 + rt])
        pw_cm.__exit__(None, None, None)

        # ---------------- column sums of blocks B1-B3 -----------------------
        with tc.tile_pool(name="cs", bufs=1, space="PSUM") as csp:
            cs_ps = csp.tile([128, 1536], f32, tag="cs")
            for rt in range(4):
                for k in (1, 2, 3):
                    nc.tensor.matmul(
                        cs_ps[0:1, (k - 1) * 512:k * 512], lhsT=ones_r[:],
                        rhs=exp_sb[:, rt * SIMW + k * 512:
                                   rt * SIMW + (k + 1) * 512],
                        start=(rt == 0), stop=(rt == 3),
                    )
            nc.scalar.copy(out=cs_sb[:, 0:512], in_=cs_ps[0:1, 0:512])
            nc.vector.tensor_copy(out=cs_sb[:, 512:1024],
                                  in_=cs_ps[0:1, 512:1024])
            nc.scalar.copy(out=cs_sb[:, 1024:1536],
                           in_=cs_ps[0:1, 1024:1536])
            nc.sync.dma_start(out=out2[:], in_=cs_sb[:])

        nc.sync.dma_start(out=out[:], in_=out_sb[:])

    nc.finalize()
    return nc


def _get_program():
    global _PROG
    if _PROG is None:
        _PROG = _build_program()
    return _PROG


def kernel(z_i, z_j, z_n, dist_labels):
    from concourse.bass_utils import run_bass_kernel_spmd

    nc = _get_program()

    z_full = np.ascontiguousarray(
        np.concatenate([z_i, z_j], axis=0), dtype=np.float32
    )
    dist = np.ascontiguousarray(dist_labels, dtype=np.float32)

    in_maps = []
    for c in range(N_CORES):
        r0 = c * RPC
        in_maps.append({
            "z": np.ascontiguousarray(np.roll(z_full, -r0, axis=0)),
            "dist": np.ascontiguousarray(np.roll(dist, -r0, axis=0)),
        })

    res = run_bass_kernel_spmd(nc, in_maps, list(range(N_CORES))).results

    S = np.zeros(N, np.float64)
    P = np.empty(N, np.float64)
    idx = np.arange(RPC)
    for c in range(N_CORES):
        o = res[c]["out"].astype(np.float64)
        g = (idx + RPC * c) % N
        # device order p*4+rt == local row 4p+rt
        S[g] += (o[:, 0:4] + o[:, 4:8] + o[:, 8:12]
                 + o[:, 12:16]).reshape(RPC)
        P[g] = o[:, 16:20].reshape(RPC)
        cs = res[c]["out2"].astype(np.float64).reshape(3, 4, 128)
        for k in (1, 2, 3):
            # block col j=kk*128+p -> local row 512k + 4p + kk
            cs_r = cs[k - 1].T.reshape(RPC)
            gk = (idx + RPC * (c + k)) % N
            S[gk] += cs_r

    S -= E_CONST
    P -= 1.0
    return np.float32((P / S).sum() / N)


# revision 73
# speedup vs baseline: 1.0002x; 1.0002x over previous
"""NT-Xent loss kernel, v20: symmetric-block decomposition (36.9us
cost-model vs 47.9us baseline; rel err ~5e-3).

v20 vs v19: sims land in FOUR independent single-buffer PSUM granule
rings per q-tile (A=b0,b1 [128,1024]; B1=b2, B2=b3, C=b4 [128,512]
each, 5 banks total) instead of two. Each granule gets its own
exp+accum into a distinct output column (host sums the four partial
row-sum groups). Finer rings decouple PE from ACT: the next strip's
sims for one 512-col block only wait for THAT block's exp, so the
exp->sims->exp serialization cycle shrinks and ACT runs ~69%% busy
through the endgame. W matmuls are issued after the chunk loop
(floor 12us) to fill the remaining PE gaps.

Decomposition: after the host-side rotation every core's q rows are
local rows 0-511; each core computes sim blocks (q, B0..B4) = local
cols 0..2559 only (5 of 8 column blocks). Row sums come from the exp
accumulators; the missing col groups m=5,6,7 for each row are COLUMN
sums of blocks B3,B2,B1 computed on cores i+5,i+6,i+7 (sim symmetry),
taken with ones-vector matmuls over the retained f32r exp values and
combined on the host. B4 pairs with core i+4's B4 (computed twice
fleet-wide, row sums only). This cuts sim matmuls 13.7->8.5us on PE
and exp work 17->11.5us on ACT vs the full-row v11 design.

Schedule/assignment (found by cost-model iteration):
- 4-rows-per-partition interleave: partition p of tile t=4ch+k holds
  local row 512ch+4p+k -> dist DMA descriptors are 1600B contiguous
  (2.28us vs 4.55us) and q/dist tiles line up for the P path.
- norms: one scalar_tensor_tensor (x*1)*x with accum_out per tile,
  all on DVE (GPSIMD rejects generic tensor ops at codegen).
- rn = exp(-0.5 ln norm2) on ACT; nn = norm2*rn on DVE.
- scale rows 0..2559 in place: ch0-1 all-Pool normalize_recip; ch2-4
  k0,k1 via one DVE broadcast-STT + k2,k3 Pool. Rows 2560+ instead
  fold rn into dist (sdist, Pool nrecip); W uses raw z there.
- transposes on PE; PSUM->SBUF copies on ACT.
- sims land in PSUM granules A=[128,1024] (b0,b1) + B=[128,1536]
  (b2-b4) per q-tile, ONE exp+accum each straight into the output
  tile; granule pools let PE run ahead of ACT.
- W = dist^T z_hat (32 matmuls) issued after the chunk loop to fill
  PE gaps during the exp phase; P path q_ps padded to 256 free
  (1 cyc/row) with the mask-multiply+reduce fused into one STT.
- colsum matmuls accumulate over the 4 q-tiles in a 3-bank PSUM
  window after the W pool closes; staged to SBUF and DMA'd as out2.
- PE warm-up matmuls ramp the tensor clock before the transposes.
- tile_set_cur_wait floors stage the scheduler per chunk arrival
  (DMA sem overhead ~0.9us included).

Engine legality learned on hardware: GPSIMD supports only its custom
ISA ops (normalize_recip etc) and cannot access PSUM; activation table
set 6 = {ln, exp, square, copy, ...}; Rsqrt/Reciprocal are banned;
f32r matmul inputs must be produced typed f32r (DVE/ACT writes
qualify); transpose-mode matmul needs a permutation rhs;
tensor_tensor_reduce crashes the exec unit.
"""

import numpy as np
from contextlib import ExitStack

N = 4096
D = 256
C = 100
B = 2048
N_CORES = 8
RPC = 512
NT = 32
NCH = 8
NBLK = 5
SIMW = NBLK * 512
NSC = NBLK * 4
E_CONST = float(np.e)

_PROG = None


def _build_program():
    import concourse.bass as bass
    import concourse.tile as tile
    from concourse import bacc, mybir, masks

    f32 = mybir.dt.float32
    f32r = mybir.dt.float32r
    MULT = mybir.AluOpType.mult
    EXP = mybir.ActivationFunctionType.Exp
    SQUARE = mybir.ActivationFunctionType.Square
    LN = mybir.ActivationFunctionType.Ln

    nc = bacc.Bacc(
        "TRN2",
        target_bir_lowering=False,
        debug=False,
        enable_asserts=False,
        num_devices=N_CORES,
    )

    z = nc.dram_tensor("z", [N, D], f32r, kind="ExternalInput").ap()
    dist = nc.dram_tensor("dist", [B, C], f32r, kind="ExternalInput").ap()
    out = nc.dram_tensor("out", [128, 20], f32, kind="ExternalOutput").ap()
    out2 = nc.dram_tensor("out2", [1, 1536], f32, kind="ExternalOutput").ap()

    with tile.TileContext(nc) as tc, ExitStack() as ctx:
        per = ctx.enter_context(tc.tile_pool(name="persist", bufs=1))

        nc.scalar.add_instruction(mybir.InstLoadActFuncSet(
            name=f"I-{nc.next_id()}", ins=[], outs=[], act_func_set_id=6))

        ident = per.tile([128, 128], f32)
        masks.make_identity(nc, ident[:])
        ident_r = per.tile([128, 128], f32r)
        nc.vector.tensor_copy(out=ident_r[:], in_=ident[:])
        ones_f = per.tile([128, 1], f32)
        nc.vector.memset(ones_f[:], 1.0)
        ones_r = per.tile([128, 1], f32r)
        nc.vector.tensor_copy(out=ones_r[:], in_=ones_f[:])

        zn = per.tile([128, NT * 256], f32r)
        zT = per.tile([128, 2 * SIMW], f32r)
        exp_sb = per.tile([128, 4 * SIMW], f32r)
        dist_sb = per.tile([128, 4 * 400], f32r)
        sdist = per.tile([128, 12 * 100], f32r)
        norm2 = per.tile([128, NT], f32)
        lnn = per.tile([128, NT], f32)
        nn = per.tile([128, NT], f32)
        rn = per.tile([128, NT], f32)
        wt_sb = per.tile([128, 256], f32)
        w_sb = per.tile([128, 2 * 256], f32r)
        junk = per.tile([128, C], f32)
        out_sb = per.tile([128, 20], f32)
        cs_sb = per.tile([1, 1536], f32)
        nc.vector.memset(w_sb[:].bitcast(f32), 0.0)

        # ---------------- DMAs: 8 z chunks, then dist ----------------------
        for ch in range(NCH):
            nc.sync.dma_start(
                out=zn[:, ch * 1024:(ch + 1) * 1024].rearrange(
                    "p (q j) -> p q j", j=256),
                in_=z[ch * 512:(ch + 1) * 512, :].rearrange(
                    "(p q) j -> p q j", q=4),
            )
        nc.sync.dma_start(
            out=dist_sb[:].rearrange("p (ch q c) -> p ch q c", q=4, c=C),
            in_=dist.rearrange("(ch p q) c -> p ch q c", p=128, q=4),
        )

        # ---------------- PE warm-up: ramp tensor clock --------------------
        with tc.tile_pool(name="warm", bufs=1, space="PSUM") as wp:
            warm = wp.tile([128, 128], f32, tag="warm")
            for _ in range(16):
                nc.tensor.matmul(warm[:], lhsT=ident_r[:], rhs=ident_r[:],
                                 start=True, stop=True)

        sqd = ctx.enter_context(tc.tile_pool(name="sqd", bufs=3))

        def arrival(ch):
            return 2.9 + 1.46 * ch

        def norms_chunk(ch):
            # ACT idles until the first rn chain; its Square backfills the
            # ch0-1 norms so the gate halves (DVE k0,k1 || ACT k2,k3).
            for k in range(4):
                t = 4 * ch + k
                src = zn[:, t * 256:(t + 1) * 256].bitcast(f32)
                s = sqd.tile([128, 256], f32, tag="sq", name=f"sq{t}")
                if ch == 0 and k == 3:
                    with tc.high_priority():
                        nc.scalar.activation(s[:], src, SQUARE,
                                             accum_out=norm2[:, t:t + 1])
                else:
                    nc.vector.scalar_tensor_tensor(
                        out=s[:], in0=src, scalar=1.0, in1=src,
                        op0=MULT, op1=MULT,
                        accum_out=norm2[:, t:t + 1])

        def rn_chain(t0, nt, need_nn):
            sl = slice(t0, t0 + nt)
            with tc.high_priority():
                nc.scalar.activation(lnn[:, sl], norm2[:, sl], LN)
                nc.scalar.activation(rn[:, sl], lnn[:, sl], EXP, scale=-0.5)
                if need_nn:
                    nc.vector.tensor_tensor(out=nn[:, sl], in0=norm2[:, sl],
                                            in1=rn[:, sl], op=MULT)

        def scale_chunk(ch):
            t0 = 4 * ch
            if ch >= 2:
                dst = zn[:, t0 * 256:(t0 + 2) * 256]
                bc = rn[:, t0:t0 + 2].unsqueeze(2).to_broadcast([128, 2, 256])
                nc.vector.scalar_tensor_tensor(
                    out=dst.rearrange("p (k j) -> p k j", j=256),
                    in0=dst.bitcast(f32).rearrange("p (k j) -> p k j", j=256),
                    scalar=1.0, in1=bc, op0=MULT, op1=MULT)
                ks = (2, 3)
            else:
                ks = (0, 1, 2, 3)
            for k in ks:
                t = 4 * ch + k
                d2 = zn[:, t * 256:(t + 1) * 256]
                nc.gpsimd.normalize_recip(
                    out_ap=d2, in_ap=d2.bitcast(f32),
                    denom_ap=nn[:, t:t + 1])

        def transpose_chunk(ptp, ch):
            for d in range(2):
                pt = ptp.tile([128, 512], f32r, tag="tr", name=f"pt{ch}_{d}")
                for k in range(4):
                    t = 4 * ch + k
                    nc.tensor.transpose(
                        pt[:, k * 128:(k + 1) * 128],
                        zn[:, t * 256 + d * 128: t * 256 + d * 128 + 128],
                        ident_r[:],
                    )
                dst = zT[:, d * SIMW + ch * 512: d * SIMW + (ch + 1) * 512]
                nc.scalar.copy(out=dst, in_=pt[:])

        def w_tiles(ts):
            for t in ts:
                if t < NSC:
                    lhsT = dist_sb[:, (t // 4 % 4) * 400 + (t % 4) * 100:
                                   (t // 4 % 4) * 400 + (t % 4) * 100 + 100]
                else:
                    lhsT = sdist[:, (t - NSC) * 100:(t - NSC + 1) * 100]
                nc.tensor.matmul(
                    wt_ps[0:C, :], lhsT=lhsT,
                    rhs=zn[:, t * 256:(t + 1) * 256],
                    start=(t == 0), stop=(t == NT - 1),
                )

        gA = ctx.enter_context(tc.tile_pool(name="gA", bufs=1, space="PSUM"))
        gB = ctx.enter_context(tc.tile_pool(name="gB", bufs=1, space="PSUM"))
        gC = ctx.enter_context(tc.tile_pool(name="gC", bufs=1, space="PSUM"))
        gB2 = ctx.enter_context(tc.tile_pool(name="gB2", bufs=1, space="PSUM"))
        pw_cm = tc.tile_pool(name="pw", bufs=1, space="PSUM")
        pw = pw_cm.__enter__()
        wt_ps = pw.tile([128, 256], f32, tag="wt", name="wt_ps")

        def sims_into(g, rt, b, col0):
            for d in range(2):
                nc.tensor.matmul(
                    g[:, col0:col0 + 512],
                    lhsT=zT[:, d * SIMW + rt * 128: d * SIMW + (rt + 1) * 128],
                    rhs=zT[:, d * SIMW + b * 512: d * SIMW + (b + 1) * 512],
                    start=(d == 0), stop=(d == 1),
                )

        GOFF = {0: 0, 1: 1024, 2: 1536, 3: 2048}

        def exp_granule(g, rt, gi, width):
            nc.scalar.activation(
                out=exp_sb[:, rt * SIMW + GOFF[gi]:
                           rt * SIMW + GOFF[gi] + width],
                in_=g[:, 0:width], func=EXP,
                accum_out=out_sb[:, gi * 4 + rt: gi * 4 + rt + 1])

        def granules_for(rt):
            a = gA.tile([128, 1024], f32, tag="gA", name=f"g{rt}a")
            sims_into(a, rt, 0, 0)
            sims_into(a, rt, 1, 512)
            exp_granule(a, rt, 0, 1024)
            bb = gB.tile([128, 512], f32, tag="gB", name=f"g{rt}b")
            sims_into(bb, rt, 2, 0)
            exp_granule(bb, rt, 1, 512)
            b2 = gB2.tile([128, 512], f32, tag="gB2", name=f"g{rt}b2")
            sims_into(b2, rt, 3, 0)
            exp_granule(b2, rt, 2, 512)
            cc = gC.tile([128, 512], f32, tag="gC", name=f"g{rt}c")
            sims_into(cc, rt, 4, 0)
            exp_granule(cc, rt, 3, 512)

        # ---------------- chunks 0-4 + strip rt0 granules -------------------
        with tc.tile_pool(name="pt", bufs=2, space="PSUM") as ptp:
            a0 = b0 = None
            for ch in range(NBLK):
                tc.tile_set_cur_wait(arrival(ch) / 1000.0)
                norms_chunk(ch)
                tc.tile_set_cur_wait((arrival(ch) + 0.40) / 1000.0)
                rn_chain(4 * ch, 4, need_nn=True)
                tc.tile_set_cur_wait((arrival(ch) + 0.75) / 1000.0)
                scale_chunk(ch)
                tc.tile_set_cur_wait((arrival(ch) + 1.30) / 1000.0)
                transpose_chunk(ptp, ch)
                tc.tile_set_cur_wait((arrival(ch) + 1.95) / 1000.0)
                if ch == 0:
                    a0 = gA.tile([128, 1024], f32, tag="gA", name="g0a")
                    sims_into(a0, 0, 0, 0)
                elif ch == 1:
                    sims_into(a0, 0, 1, 512)
                    exp_granule(a0, 0, 0, 1024)
                elif ch == 2:
                    b0 = gB.tile([128, 512], f32, tag="gB", name="g0b")
                    sims_into(b0, 0, 2, 0)
                    exp_granule(b0, 0, 1, 512)
                elif ch == 3:
                    b02 = gB2.tile([128, 512], f32, tag="gB2", name="g0b2")
                    sims_into(b02, 0, 3, 0)
                    exp_granule(b02, 0, 2, 512)
                else:
                    c0 = gC.tile([128, 512], f32, tag="gC", name="g0c")
                    sims_into(c0, 0, 4, 0)
                    exp_granule(c0, 0, 3, 512)

        for ch in range(NBLK, NCH):
            tc.tile_set_cur_wait(arrival(ch) / 1000.0)
            norms_chunk(ch)
        tc.tile_set_cur_wait((arrival(7) + 0.40) / 1000.0)
        rn_chain(NSC, 12, need_nn=True)
        for t in range(NSC, NT):
            srcd = dist_sb[:, (t // 4 % 4) * 400 + (t % 4) * 100:
                           (t // 4 % 4) * 400 + (t % 4) * 100 + 100]
            nc.gpsimd.normalize_recip(
                out_ap=sdist[:, (t - NSC) * 100:(t - NSC + 1) * 100],
                in_ap=srcd.bitcast(f32),
                denom_ap=nn[:, t:t + 1])

        # ---------------- strips 1-3, W tail, P path ------------------------
        tc.tile_set_cur_wait(12.0 / 1000.0)
        w_tiles(range(0, NSC))
        granules_for(1)
        tc.tile_set_cur_wait((arrival(7) + 1.5) / 1000.0)
        w_tiles(range(NSC, NT))
        granules_for(2)

        nc.vector.tensor_copy(out=wt_sb[0:C, :], in_=wt_ps[0:C, :])
        for d in range(2):
            w_ps = pw.tile([128, 128], f32, tag="wt", name=f"w_ps{d}")
            nc.tensor.transpose(
                w_ps[:, 0:C],
                wt_sb[0:C, d * 128:(d + 1) * 128],
                ident[0:C, 0:C],
            )
            nc.vector.tensor_copy(out=w_sb[:, d * 256:d * 256 + C],
                                  in_=w_ps[:, 0:C])

        cc3 = gC.tile([128, 512], f32, tag="gC", name="g3c")
        sims_into(cc3, 3, 4, 0)
        exp_granule(cc3, 3, 3, 512)
        bb3 = gB.tile([128, 512], f32, tag="gB", name="g3b")
        sims_into(bb3, 3, 2, 0)
        exp_granule(bb3, 3, 1, 512)
        b32 = gB2.tile([128, 512], f32, tag="gB2", name="g3b2")
        sims_into(b32, 3, 3, 0)
        exp_granule(b32, 3, 2, 512)
        a3 = gA.tile([128, 1024], f32, tag="gA", name="g3a")
        sims_into(a3, 3, 0, 0)
        sims_into(a3, 3, 1, 512)
        exp_granule(a3, 3, 0, 1024)

        for rt in range(4):
            q_ps = pw.tile([128, 256], f32, tag="wt", name=f"q_ps{rt}")
            for d in range(2):
                nc.tensor.matmul(
                    q_ps[:], lhsT=zT[:, d * SIMW + rt * 128:
                                     d * SIMW + (rt + 1) * 128],
                    rhs=w_sb[:, d * 256:(d + 1) * 256],
                    start=(d == 0), stop=(d == 1),
                )
            nc.vector.scalar_tensor_tensor(
                out=junk[:], in0=q_ps[:, 0:C], scalar=1.0,
                in1=dist_sb[:, rt * 100:rt * 100 + 100].bitcast(f32),
                op0=MULT, op1=MULT,
                accum_out=out_sb[:, 16 + rt:17 + rt])
        pw_cm.__exit__(None, None, None)

        # ---------------- column sums of blocks B1-B3 -----------------------
        with tc.tile_pool(name="cs", bufs=1, space="PSUM") as csp:
            cs_ps = csp.tile([128, 1536], f32, tag="cs")
            for rt in range(4):
                for k in (1, 2, 3):
                    nc.tensor.matmul(
                        cs_ps[0:1, (k - 1) * 512:k * 512], lhsT=ones_r[:],
                        rhs=exp_sb[:, rt * SIMW + k * 512:
                                   rt * SIMW + (k + 1) * 512],
                        start=(rt == 0), stop=(rt == 3),
                    )
            nc.scalar.copy(out=cs_sb[:, 0:512], in_=cs_ps[0:1, 0:512])
            nc.vector.tensor_copy(out=cs_sb[:, 512:1024],
                                  in_=cs_ps[0:1, 512:1024])
            nc.scalar.copy(out=cs_sb[:, 1024:1536],
                           in_=cs_ps[0:1, 1024:1536])
            nc.sync.dma_start(out=out2[:], in_=cs_sb[:])

        nc.sync.dma_start(out=out[:], in_=out_sb[:])

    nc.finalize()
    return nc


def _get_program():
    global _PROG
    if _PROG is None:
        _PROG = _build_program()
    return _PROG


def kernel(z_i, z_j, z_n, dist_labels):
    from concourse.bass_utils import run_bass_kernel_spmd

    nc = _get_program()

    z_full = np.ascontiguousarray(
        np.concatenate([z_i, z_j], axis=0), dtype=np.float32
    )
    dist = np.ascontiguousarray(dist_labels, dtype=np.float32)

    in_maps = []
    for c in range(N_CORES):
        r0 = c * RPC
        in_maps.append({
            "z": np.ascontiguousarray(np.roll(z_full, -r0, axis=0)),
            "dist": np.ascontiguousarray(np.roll(dist, -r0, axis=0)),
        })

    res = run_bass_kernel_spmd(nc, in_maps, list(range(N_CORES))).results

    S = np.zeros(N, np.float64)
    P = np.empty(N, np.float64)
    idx = np.arange(RPC)
    for c in range(N_CORES):
        o = res[c]["out"].astype(np.float64)
        g = (idx + RPC * c) % N
        # device order p*4+rt == local row 4p+rt
        S[g] += (o[:, 0:4] + o[:, 4:8] + o[:, 8:12]
                 + o[:, 12:16]).reshape(RPC)
        P[g] = o[:, 16:20].reshape(RPC)
        cs = res[c]["out2"].astype(np.float64).reshape(3, 4, 128)
        for k in (1, 2, 3):
            # block col j=kk*128+p -> local row 512k + 4p + kk
            cs_r = cs[k - 1].T.reshape(RPC)
            gk = (idx + RPC * (c + k)) % N
            S[gk] += cs_r

    S -= E_CONST
    P -= 1.0
    return np.float32((P / S).sum() / N)
